# revision 2
# baseline (speedup 1.0000x reference)
"""Dynamic-weight conv2d (DYDConv2d) Trainium2 kernel — fp8 DoubleRow version.

Problem: per-sample SE-gated mixture of K=4 conv filter banks, then a 3x3
conv (pad 1) with the per-sample aggregated weights.

  pooled = mean_hw(x)                     [B, C]
  h      = relu(pooled @ fc1_w.T)         [B, 65]
  y      = h @ fc2_w.T + fc2_b            [B, 1024]
  prob   = softmax(y.reshape(B,4,256)/30) [B, 4, 256]
  agg    = einsum('bko,kof->bof', prob, W.reshape(4,256,2304))
  out[b] = conv2d(x[b], agg[b].reshape(256,256,3,3), pad=1)

Sharding: pure data-parallel over batch. 8 cores x 2 samples each; every
core holds the full filter bank + SE params. No cross-core comm.

Per-core plan: the conv runs as fp8e4 DoubleRow matmuls (2 ci-blocks
contracted per pass at 0.5 cycles/row). Precision is recovered with a
hi/lo split of both operands:
  x   ~ x_hi + x_lo        (x_lo = fp8 of the fp8-rounding residual)
  32w ~ w_hi + w_lo
  out = [w_hi@x_hi + w_hi@x_lo + w_lo@x_hi] / 32   (w_lo@x_lo dropped)
Three accumulating matmul sets per psum bank; rel err ~2e-3 (better than
the bf16 direct conv) at 3/4 of half the bf16 matmul cost.

DoubleRow rhs must be a rank-3 AP [128, 2, N]; the padded x layout
[128, 2ci_blk, 66, 68] provides [128, 2, 64] row slices, so the conv
issues one matmul per (output row, offset, set): 216 per psum bank.

DMA queue order: fc1 -> x(s0) -> fc2 -> W(ob0) -> W(ob1) -> x(s1) -> outs.
"""
import sys

for _p in ("/opt/trn_rl_repo", "/root/.axon_site/_ro/trn_rl_repo"):
    if _p not in sys.path:
        sys.path.insert(0, _p)

import numpy as np

try:  # persistent jax compile cache: makes repeat invocations fast
    import jax
    jax.config.update("jax_compilation_cache_dir", "/tmp/jaxcache")
except Exception:
    pass

import concourse.bass as bass
import concourse.tile as tile
from concourse import bacc, mybir
from concourse.bass_utils import run_bass_kernel_spmd
from concourse.masks import make_identity

F32 = mybir.dt.float32
BF16 = mybir.dt.bfloat16
FP8 = mybir.dt.float8e4
MULT = mybir.AluOpType.mult
ADD = mybir.AluOpType.add
SUB = mybir.AluOpType.subtract
ACT_COPY = mybir.ActivationFunctionType.Copy
ACT_RELU = mybir.ActivationFunctionType.Relu
ACT_EXP = mybir.ActivationFunctionType.Exp
DR = mybir.MatmulPerfMode.DoubleRow

B, C, H, W = 16, 256, 64, 64
O, K, HID = 256, 4, 65
KK = 3  # kernel spatial size
NOFF = KK * KK  # 9
CF = C * NOFF  # 2304  (ci, off) flattened
N_CORES = 8
BS = B // N_CORES  # samples per core
TEMP = 30.0
WSCALE = 32.0  # weight pre-scale so 32*agg sits in e4m3's normal range
# padded x layout: row stride 68 (left pad 2 keeps 4B alignment), 66 rows
PH, PW = H + 2, 68
HWCHUNKS = (1536, 1536, 512, 512)  # free-dim chunking of the 4096 out pixels
TGROUPS = ((0, 4), (4, 8), (8, 9))  # transpose off-batches
# (w_set, x_set) product pairs: hi*hi, hi*lo, lo*hi
PRODUCTS = ((0, 0), (0, 1), (1, 0))


def build_kernel():
    nc = bacc.Bacc("TRN2", target_bir_lowering=False, debug=False,
                   num_devices=N_CORES)
    x_d = nc.dram_tensor("x", [BS, C, H, W], F32, kind="ExternalInput")
    fc1_d = nc.dram_tensor("fc1_w", [HID, C], F32, kind="ExternalInput")
    fc2_d = nc.dram_tensor("fc2_w", [K * O, HID], F32, kind="ExternalInput")
    fc2b_d = nc.dram_tensor("fc2_b", [K * O], F32, kind="ExternalInput")
    w_d = nc.dram_tensor("weight", [K, O, C, KK, KK], F32, kind="ExternalInput")
    out_d = nc.dram_tensor("out", [BS, O, H, W], F32, kind="ExternalOutput")

    with tile.TileContext(nc) as tc:
        _body(nc, tc, x_d, fc1_d, fc2_d, fc2b_d, w_d, out_d)
    nc.compile()
    return nc


def _body(nc, tc, x_d, fc1_d, fc2_d, fc2b_d, w_d, out_d):
    with (
        tc.tile_pool(name="const", bufs=1) as constp,
        tc.tile_pool(name="wbank", bufs=1) as wbank,
        tc.tile_pool(name="wstage", bufs=7) as wstage,
        tc.tile_pool(name="xq", bufs=8) as xqp,
        tc.tile_pool(name="xb", bufs=1) as xbp,
        tc.tile_pool(name="aggp", bufs=2) as aggp,
        tc.tile_pool(name="aggtp", bufs=1) as aggtp,
        tc.tile_pool(name="small", bufs=2) as smallp,
        tc.tile_pool(name="ost", bufs=3) as ostp,
        tc.tile_pool(name="psc", bufs=2, space=bass.MemorySpace.PSUM) as pscp,
        tc.tile_pool(name="pst", bufs=2, space=bass.MemorySpace.PSUM) as pstp,
    ):
        with nc.named_scope("params"):
            ident = constp.tile([128, 128], BF16)
            make_identity(nc, ident[:])
            ident32 = constp.tile([128, 128], F32)
            make_identity(nc, ident32[:])

        # padded fp8 x tiles: [ci_in_blk, ci_blk, ph, pw], hi and lo
        xb = [[xbp.tile([128, 2, PH, PW], FP8, name=f"xb{s}_{v}")
               for v in range(2)] for s in range(BS)]
        pooled, se = [], []
        zcols = [[(e // 4, e) for e in range(8)]] * 2

        def halo_memset(s):
            for v in range(2):
                for blk in range(2):
                    t = xb[s][v]
                    nc.gpsimd.memset(t[:, blk, 0, :], 0.0)
                    nc.gpsimd.memset(t[:, blk, PH - 1, :], 0.0)
                    nc.gpsimd.memset(t[:, blk, 0:PH - 1, PW - 2:PW], 0.0)
                    nc.gpsimd.memset(t[:, blk, 1:PH, 0:2], 0.0)

        def xload_dma(s):
            """Eight 16-row eighth DMAs covering both ci-blocks of x[s] —
            small chunks shorten the pooled tail (last cast is short)."""
            with nc.named_scope(f"xload{s}"):
                pooled.append(smallp.tile([128, 8], F32, tag="pooled",
                                          name=f"pooled{s}"))
                tiles = []
                for e in range(8):
                    blk, hh = e // 4, e % 4
                    xq = xqp.tile([128, H // 4, W], F32, tag="xq",
                                  name=f"xq{s}_{e}")
                    nc.sync.dma_start(
                        xq[:], x_d[s, blk * 128:(blk + 1) * 128,
                                   hh * 16:(hh + 1) * 16])
                    tiles.append(xq)
                return tiles

        def xload_cast(s, tiles):
            """hi cast (fused pooled accumulation) + lo residual.
            s0: hi on ACT, lo on DVE (conv needs them early);
            s1: hi on DVE, lo on the otherwise-idle gpsimd."""
            with nc.named_scope(f"xcast{s}"):
                for e, xq in enumerate(tiles):
                    blk, hh = e // 4, e % 4
                    r0 = 1 + 16 * hh
                    hi = xb[s][0][:, blk, r0:r0 + 16, 2:W + 2]
                    lo = xb[s][1][:, blk, r0:r0 + 16, 2:W + 2]
                    if s == 0:
                        nc.scalar.activation(
                            hi, xq[:], ACT_COPY,
                            accum_out=pooled[s][:, e:e + 1])
                        nc.vector.tensor_tensor(lo, xq[:], hi, SUB)
                    else:
                        nc.vector.tensor_scalar(
                            hi, xq[:], 1.0, None, MULT, ADD,
                            accum_out=pooled[s][:, e:e + 1])
                        nc.gpsimd.tensor_tensor(lo, xq[:], hi, SUB)

        def params_rest():
            with nc.named_scope("params"):
                fc1n = constp.tile([128, C], F32)  # rows 0..64 = fc1_w
                nc.sync.dma_start(fc1n[0:HID, :], fc1_d[:])
                fc2n = constp.tile([128, 8, HID], F32)  # [i_in_blk, i_blk, j]
                nc.sync.dma_start(
                    fc2n[:], bass.AP(fc2_d, 0, [[HID, 128], [128 * HID, 8],
                                                [1, HID]]))
                fc1t = constp.tile([128, 2, HID], F32)  # [ci_in_blk, ci_blk, j]
                for blk in range(2):
                    tps = pstp.tile([128, HID], F32, tag="pt", name=f"tp1_{blk}")
                    nc.tensor.transpose(tps[:],
                                        fc1n[0:HID, blk * 128:(blk + 1) * 128],
                                        ident32[0:HID, 0:HID])
                    nc.scalar.copy(fc1t[:, blk, :], tps[:])
                fc2t = constp.tile([128, K * O], F32)  # unused rows 66..127
                # rows 0..64 = fc2_w.T ; row 65 = fc2_b (bias folded in)
                for half in range(2):
                    tps = pstp.tile([128, 512], F32, tag="pt",
                                    name=f"tp2_{half}")
                    for c in range(4):
                        nc.tensor.transpose(tps[0:HID, c * 128:(c + 1) * 128],
                                            fc2n[:, half * 4 + c, :],
                                            ident32[:])
                    nc.scalar.copy(fc2t[0:HID, half * 512:(half + 1) * 512],
                                   tps[0:HID, :])
                nc.sync.dma_start(fc2t[HID:HID + 1, :], fc2b_d[:].unsqueeze(0))
                return fc1t, fc2t

        def se_chain(s, fc1t, fc2t):
            with nc.named_scope(f"se{s}"):
                z_ps = pstp.tile([128, 1], F32, tag="pt", name=f"z{s}")
                cols = zcols[s]
                for i, (blk, col) in enumerate(cols):
                    nc.tensor.matmul(z_ps[0:HID, :], fc1t[:, blk, :],
                                     pooled[s][:, col:col + 1],
                                     start=(i == 0), stop=(i == len(cols) - 1))
                h_ext = smallp.tile([128, 1], F32, tag="hext", name=f"hext{s}")
                nc.vector.memset(h_ext[:], 1.0)  # row 65 stays 1.0 (bias row)
                # relu(z/4096): mean folded via scale (relu is scale-invariant)
                nc.scalar.activation(h_ext[0:HID, :], z_ps[0:HID, :], ACT_RELU,
                                     scale=1.0 / (H * W))
                y_ps = pstp.tile([128, K * 2], F32, tag="pt", name=f"y{s}")
                for c in range(K * 2):
                    nc.tensor.matmul(y_ps[:, c:c + 1],
                                     fc2t[0:HID + 1, c * 128:(c + 1) * 128],
                                     h_ext[0:HID + 1, :], start=True, stop=True)
                e = smallp.tile([128, K, 2], F32, tag="e", name=f"e{s}")
                nc.scalar.activation(e[:].rearrange("p a b -> p (a b)"),
                                     y_ps[:], ACT_EXP, scale=1.0 / TEMP)
                ssum = smallp.tile([128, 2], F32, tag="ssum", name=f"ssum{s}")
                er = e[:].rearrange("p k o -> p o k")
                nc.vector.tensor_reduce(ssum[:], er, mybir.AxisListType.X, ADD)
                rinv = smallp.tile([128, 2], F32, tag="rinv", name=f"rinv{s}")
                nc.vector.reciprocal(rinv[:], ssum[:])
                prob = smallp.tile([128, 2, K], F32, tag="prob", name=f"prob{s}")
                for ob in range(2):
                    nc.vector.tensor_scalar_mul(prob[:, ob], er[:, ob],
                                                rinv[:, ob:ob + 1])
                return prob

        wb = [wbank.tile([128, K, C, NOFF], BF16, name=f"wb{ob}")
              for ob in range(2)]

        def load_w_dma(ob):
            # ci-half-major chunks so agg/transposes for ci-block 0 can
            # start while ci-block 1 is still in flight on the DMA ring.
            # Returns the f32 staging tiles: sample 0's agg reads them
            # directly (skips the cast latency on its critical path); the
            # bf16 wb copy only feeds sample 1's agg.
            staged = {}
            with nc.named_scope(f"wload{ob}"):
                for cb in range(2):
                    for k in range(K):
                        wst = wstage.tile([128, CF // 2], F32, tag="wst")
                        nc.sync.dma_start(
                            wst[:],
                            w_d[k, ob * 128:(ob + 1) * 128,
                                cb * 128:(cb + 1) * 128].rearrange(
                                    "p c a b -> p (c a b)"))
                        staged[(cb, k)] = wst
            return staged

        def cast_w(ob, staged):
            # wb only feeds sample 1's agg (late deadline): cast on the
            # idle gpsimd, except the first group which fits on ACT early.
            with nc.named_scope(f"wcast{ob}"):
                for cb in range(2):
                    for k in range(K):
                        dst = wb[ob][:, k, cb * 128:(cb + 1) * 128,
                                     :].rearrange("p c o -> p (c o)")
                        if ob == 0 and cb == 0:
                            nc.scalar.copy(dst, staged[(cb, k)][:])
                        else:
                            nc.gpsimd.tensor_copy(dst, staged[(cb, k)][:])

        # agg (bf16, [o, ci, off]) on DVE; emitted separately from the PE
        # transposes + hi/lo fp8 quant (aggt[set] [ci_in_blk, off, ci_blk, o])
        # so the DVE chain pipelines with the W DMA chunk arrivals and the
        # transpose/quant latency hides under unrelated conv matmuls.
        def agg_compute(s, ob, agg, staged=None):
            with nc.named_scope(f"agg{s}_{ob}"):
                for cb in range(2):
                    asl = agg[ob][:, cb * 128:(cb + 1) * 128,
                                  :].rearrange("p c o -> p (c o)")
                    for k in range(K):
                        if staged is not None:  # f32 staging (sample 0)
                            src = staged[(cb, k)][:]
                        else:  # resident bf16 bank (sample 1)
                            src = wb[ob][:, k, cb * 128:(cb + 1) * 128,
                                         :].rearrange("p c o -> p (c o)")
                        sc = se[s][:, ob, k:k + 1]
                        if k == 0:
                            nc.vector.tensor_scalar_mul(asl, src, sc)
                        else:
                            nc.vector.scalar_tensor_tensor(asl, src, sc,
                                                           asl, MULT, ADD)

        def agg_transpose(s, ob, agg, aggt):
            with nc.named_scope(f"transp{s}_{ob}"):
                for cb in range(2):
                    for gi, (o0, o1) in enumerate(TGROUPS):
                        n = o1 - o0
                        pt = pstp.tile([128, 4, 128], BF16, tag="pt",
                                       name=f"pt{s}_{ob}_{cb}_{gi}")
                        for oi in range(n):
                            nc.tensor.transpose(
                                pt[:, oi, :],
                                agg[ob][:, cb * 128:(cb + 1) * 128, o0 + oi],
                                ident[:])
                        src = pt[:, 0:n, :]
                        dst_hi = aggt[0][:, o0:o1, cb, ob * 128:(ob + 1) * 128]
                        dst_lo = aggt[1][:, o0:o1, cb, ob * 128:(ob + 1) * 128]
                        nc.scalar.activation(dst_hi, src, ACT_COPY,
                                             scale=WSCALE)
                        nc.vector.scalar_tensor_tensor(dst_lo, src, WSCALE,
                                                       dst_hi, MULT, SUB)

        def agg_transpose2(s, ob, agg, aggt):
            """Variant: one 8-off transpose batch + one single-off batch per
            ci-block -> only 2 hi + 2 lo quant ops per (s, ob, cb)."""
            with nc.named_scope(f"transp{s}_{ob}"):
                for cb in range(2):
                    for gi, (o0, o1) in enumerate(((0, 8), (8, 9))):
                        n = o1 - o0
                        pt = pstp.tile([128, 8, 128], BF16, tag="pt",
                                       name=f"pt{s}_{ob}_{cb}_{gi}")
                        for oi in range(n):
                            nc.tensor.transpose(
                                pt[:, oi, :],
                                agg[ob][:, cb * 128:(cb + 1) * 128, o0 + oi],
                                ident[:])
                        src = pt[:, 0:n, :]
                        dst_hi = aggt[0][:, o0:o1, cb, ob * 128:(ob + 1) * 128]
                        dst_lo = aggt[1][:, o0:o1, cb, ob * 128:(ob + 1) * 128]
                        nc.scalar.activation(dst_hi, src, ACT_COPY,
                                             scale=WSCALE)
                        nc.vector.scalar_tensor_tensor(dst_lo, src, WSCALE,
                                                       dst_hi, MULT, SUB)

        # DoubleRow pairings: four within-ci-block OFF pairs (so those
        # matmuls depend on only one ci-block's aggt) + off8 paired across
        # ci-blocks. Same matmul count/cost as pure cb-pairing, but conv can
        # start as soon as ci-block 0's weights are quantized.
        OFFPAIRS = ((0, 1), (3, 4), (6, 7), (2, 5))

        def strides_of(ap):
            apl = [list(p) for p in ap.ap]
            return apl

        def conv(s, ob, aggt, mid=None):
            out_hw = out_d[s].rearrange("o a b -> o (a b)")
            xfull = [xb[s][v][:] for v in range(2)]
            xst = [strides_of(a) for a in xfull]  # [[sp,128],[scb,2],[sr,PH],[sc,PW]]
            afull = [aggt[v][:] for v in range(2)]
            ast = [strides_of(a) for a in afull]  # [[sp,128],[soff,9],[scb,2],[so,256]]
            with nc.named_scope(f"conv{s}_{ob}"):
                c0 = 0
                for ci, csz in enumerate(HWCHUNKS):
                    if ci == 2 and mid is not None:
                        mid()
                    pc = pscp.tile([128, max(HWCHUNKS)], F32, tag="conv",
                                   name=f"conv{s}_{ob}_{ci}")
                    for b in range(csz // 512):
                        h0 = (c0 + b * 512) // W
                        n_mm = 0
                        # cb-major then set-major: cb0 hi*hi first
                        for cb in range(2):
                            for (ws, xs_i) in PRODUCTS:
                                sp, scb, sr, sc = (xst[xs_i][0][0],
                                                   xst[xs_i][1][0],
                                                   xst[xs_i][2][0],
                                                   xst[xs_i][3][0])
                                asp, soff, sacb, so = (ast[ws][0][0],
                                                       ast[ws][1][0],
                                                       ast[ws][2][0],
                                                       ast[ws][3][0])
                                for (o1, o2) in OFFPAIRS:
                                    dh1, dw1 = o1 // KK - 1, o1 % KK - 1
                                    dh2, dw2 = o2 // KK - 1, o2 % KK - 1
                                    delta = (dh2 - dh1) * sr + (dw2 - dw1) * sc
                                    lhsT = bass.AP(
                                        afull[ws].tensor,
                                        afull[ws].offset + o1 * soff
                                        + cb * sacb + ob * 128 * so,
                                        [[asp, 128], [(o2 - o1) * soff, 2],
                                         [so, 128]])
                                    for r in range(8):
                                        roff = (xfull[xs_i].offset + cb * scb
                                                + (h0 + r + 1 + dh1) * sr
                                                + (2 + dw1) * sc)
                                        rhs = bass.AP(
                                            xfull[xs_i].tensor, roff,
                                            [[sp, 128], [delta, 2], [sc, W]])
                                        o0 = b * 512 + r * 64
                                        nc.tensor.matmul(
                                            pc[:, o0:o0 + 64], lhsT, rhs,
                                            start=(n_mm == 0),
                                            stop=False, perf_mode=DR)
                                        n_mm += 1
                        # off8 (dh=+1, dw=+1): pair the two ci-blocks
                        for (ws, xs_i) in PRODUCTS:
                            xt = xb[s][xs_i]
                            lhsT = aggt[ws][:, 8, :, ob * 128:(ob + 1) * 128]
                            for r in range(8):
                                rhs = xt[:, :, h0 + r + 2, 3:3 + W]
                                o0 = b * 512 + r * 64
                                nc.tensor.matmul(
                                    pc[:, o0:o0 + 64], lhsT, rhs,
                                    start=False,
                                    stop=(n_mm == 27 * 8 - 1),
                                    perf_mode=DR)
                                n_mm += 1
                    ost = ostp.tile([128, max(HWCHUNKS)], F32, tag="ost")
                    nc.scalar.activation(ost[:, 0:csz], pc[:, 0:csz],
                                         ACT_COPY, scale=1.0 / WSCALE)
                    nc.sync.dma_start(
                        out_hw[ob * 128:(ob + 1) * 128, c0:c0 + csz],
                        ost[:, 0:csz])
                    c0 += csz

        # ---- emission: DMA order x_s0, fc1, fc2, W0, W1, x_s1, outs ------
        # s1's transposes/quant are emitted mid-way through the preceding
        # conv block so their latency hides under ready conv matmuls.
        halo_memset(0)
        xq0 = xload_dma(0)
        xload_cast(0, xq0)
        fc1t, fc2t = params_rest()
        agg0 = [aggp.tile([128, C, NOFF], BF16, tag="agg", name=f"agg0_{ob}")
                for ob in range(2)]
        aggt0 = [aggtp.tile([128, NOFF, 2, O], FP8, name=f"aggt0_{v}")
                 for v in range(2)]
        with tc.high_priority():
            se.append(se_chain(0, fc1t, fc2t))
        wst0 = load_w_dma(0)
        agg_compute(0, 0, agg0, wst0)
        agg_transpose(0, 0, agg0, aggt0)
        cast_w(0, wst0)
        wst1 = load_w_dma(1)
        agg_compute(0, 1, agg0, wst1)
        agg_transpose(0, 1, agg0, aggt0)
        cast_w(1, wst1)
        halo_memset(1)
        xq1 = xload_dma(1)
        xload_cast(1, xq1)
        se.append(se_chain(1, fc1t, fc2t))
        agg1 = [aggp.tile([128, C, NOFF], BF16, tag="agg", name=f"agg1_{ob}")
                for ob in range(2)]
        aggt1 = [aggtp.tile([128, NOFF, 2, O], FP8, name=f"aggt1_{v}")
                 for v in range(2)]
        agg_compute(1, 0, agg1)
        agg_compute(1, 1, agg1)
        conv(0, 0, aggt0)
        conv(0, 1, aggt0, mid=lambda: agg_transpose(1, 0, agg1, aggt1))
        conv(1, 0, aggt1, mid=lambda: agg_transpose(1, 1, agg1, aggt1))
        conv(1, 1, aggt1)


_NC_CACHE = None


def _get_nc():
    global _NC_CACHE
    if _NC_CACHE is None:
        _NC_CACHE = build_kernel()
    return _NC_CACHE


def make_in_maps(x, fc1_w, fc2_w, fc2_b, weight):
    x = np.ascontiguousarray(x, dtype=np.float32)
    shared = {
        "fc1_w": np.ascontiguousarray(fc1_w, dtype=np.float32),
        "fc2_w": np.ascontiguousarray(fc2_w, dtype=np.float32),
        "fc2_b": np.ascontiguousarray(fc2_b, dtype=np.float32),
        "weight": np.ascontiguousarray(weight, dtype=np.float32),
    }
    return [{"x": x[c * BS:(c + 1) * BS], **shared} for c in range(N_CORES)]


def kernel(x, fc1_w, fc2_w, fc2_b, weight):
    import time
    nc = _get_nc()
    in_maps = make_in_maps(x, fc1_w, fc2_w, fc2_b, weight)
    res = None
    for attempt in range(3):
        try:
            res = run_bass_kernel_spmd(nc, in_maps,
                                       core_ids=list(range(N_CORES)))
            break
        except Exception:
            # transient device wedge (NRT_EXEC_UNIT_UNRECOVERABLE); the
            # axon terminal recovers after a short wait
            if attempt == 2:
                raise
            time.sleep(60 * (attempt + 1))
    return np.concatenate([res.results[c]["out"] for c in range(N_CORES)],
                          axis=0).astype(np.float32)


# revision 4
# speedup vs baseline: 1.0025x; 1.0025x over previous
"""Dynamic-weight conv2d (DYDConv2d) Trainium2 kernel — fp8 DoubleRow version.

Problem: per-sample SE-gated mixture of K=4 conv filter banks, then a 3x3
conv (pad 1) with the per-sample aggregated weights.

  pooled = mean_hw(x)                     [B, C]
  h      = relu(pooled @ fc1_w.T)         [B, 65]
  y      = h @ fc2_w.T + fc2_b            [B, 1024]
  prob   = softmax(y.reshape(B,4,256)/30) [B, 4, 256]
  agg    = einsum('bko,kof->bof', prob, W.reshape(4,256,2304))
  out[b] = conv2d(x[b], agg[b].reshape(256,256,3,3), pad=1)

Sharding: pure data-parallel over batch. 8 cores x 2 samples each; every
core holds the full filter bank + SE params. No cross-core comm.

Per-core plan: the conv runs as fp8e4 DoubleRow matmuls (2 ci-blocks
contracted per pass at 0.5 cycles/row). Precision is recovered with a
hi/lo split of both operands:
  x   ~ x_hi + x_lo        (x_lo = fp8 of the fp8-rounding residual)
  32w ~ w_hi + w_lo
  out = [w_hi@x_hi + w_hi@x_lo + w_lo@x_hi] / 32   (w_lo@x_lo dropped)
Three accumulating matmul sets per psum bank; rel err ~2e-3 (better than
the bf16 direct conv) at 3/4 of half the bf16 matmul cost.

DoubleRow rhs must be a rank-3 AP [128, 2, N]; the padded x layout
[128, 2ci_blk, 66, 68] provides [128, 2, 64] row slices, so the conv
issues one matmul per (output row, offset, set): 216 per psum bank.

DMA queue order: fc1 -> x(s0) -> fc2 -> W(ob0) -> W(ob1) -> x(s1) -> outs.
"""
import sys

for _p in ("/opt/trn_rl_repo", "/root/.axon_site/_ro/trn_rl_repo"):
    if _p not in sys.path:
        sys.path.insert(0, _p)

import numpy as np

try:  # persistent jax compile cache: makes repeat invocations fast
    import jax
    jax.config.update("jax_compilation_cache_dir", "/tmp/jaxcache")
except Exception:
    pass

import concourse.bass as bass
import concourse.tile as tile
from concourse import bacc, mybir
from concourse.bass_utils import run_bass_kernel_spmd
from concourse.masks import make_identity

F32 = mybir.dt.float32
BF16 = mybir.dt.bfloat16
FP8 = mybir.dt.float8e4
MULT = mybir.AluOpType.mult
ADD = mybir.AluOpType.add
SUB = mybir.AluOpType.subtract
ACT_COPY = mybir.ActivationFunctionType.Copy
ACT_RELU = mybir.ActivationFunctionType.Relu
ACT_EXP = mybir.ActivationFunctionType.Exp
DR = mybir.MatmulPerfMode.DoubleRow

B, C, H, W = 16, 256, 64, 64
O, K, HID = 256, 4, 65
KK = 3  # kernel spatial size
NOFF = KK * KK  # 9
CF = C * NOFF  # 2304  (ci, off) flattened
N_CORES = 8
BS = B // N_CORES  # samples per core
TEMP = 30.0
WSCALE = 32.0  # weight pre-scale so 32*agg sits in e4m3's normal range
# padded x layout: row stride 68 (left pad 2 keeps 4B alignment), 66 rows
PH, PW = H + 2, 68
HWCHUNKS = (1536, 1536, 512, 512)  # free-dim chunking of the 4096 out pixels
TGROUPS = ((0, 4), (4, 8), (8, 9))  # transpose off-batches
# (w_set, x_set) product pairs: hi*hi, hi*lo, lo*hi
PRODUCTS = ((0, 0), (0, 1), (1, 0))


def build_kernel():
    nc = bacc.Bacc("TRN2", target_bir_lowering=False, debug=False,
                   num_devices=N_CORES)
    x_d = nc.dram_tensor("x", [BS, C, H, W], F32, kind="ExternalInput")
    fc1_d = nc.dram_tensor("fc1_w", [HID, C], F32, kind="ExternalInput")
    fc2_d = nc.dram_tensor("fc2_w", [K * O, HID], F32, kind="ExternalInput")
    fc2b_d = nc.dram_tensor("fc2_b", [K * O], F32, kind="ExternalInput")
    w_d = nc.dram_tensor("weight", [K, O, C, KK, KK], F32, kind="ExternalInput")
    out_d = nc.dram_tensor("out", [BS, O, H, W], F32, kind="ExternalOutput")

    with tile.TileContext(nc) as tc:
        _body(nc, tc, x_d, fc1_d, fc2_d, fc2b_d, w_d, out_d)
    nc.compile()
    return nc


def _body(nc, tc, x_d, fc1_d, fc2_d, fc2b_d, w_d, out_d):
    with (
        tc.tile_pool(name="const", bufs=1) as constp,
        tc.tile_pool(name="wbank", bufs=1) as wbank,
        tc.tile_pool(name="wstage", bufs=8) as wstage,
        tc.tile_pool(name="xq", bufs=8) as xqp,
        tc.tile_pool(name="xb", bufs=1) as xbp,
        tc.tile_pool(name="aggp", bufs=2) as aggp,
        tc.tile_pool(name="aggtp", bufs=1) as aggtp,
        tc.tile_pool(name="small", bufs=2) as smallp,
        tc.tile_pool(name="ost", bufs=4) as ostp,
        tc.tile_pool(name="psc", bufs=2, space=bass.MemorySpace.PSUM) as pscp,
        tc.tile_pool(name="pst", bufs=2, space=bass.MemorySpace.PSUM) as pstp,
    ):
        with nc.named_scope("params"):
            ident = constp.tile([128, 128], BF16)
            make_identity(nc, ident[:])
            ident32 = constp.tile([128, 128], F32)
            make_identity(nc, ident32[:])

        # padded fp8 x tiles: [ci_in_blk, ci_blk, ph, pw], hi and lo
        xb = [[xbp.tile([128, 2, PH, PW], FP8, name=f"xb{s}_{v}")
               for v in range(2)] for s in range(BS)]
        pooled, se = [], []
        zcols = [[(e // 4, e) for e in range(8)]] * 2

        def halo_memset(s):
            for v in range(2):
                for blk in range(2):
                    t = xb[s][v]
                    nc.gpsimd.memset(t[:, blk, 0, :], 0.0)
                    nc.gpsimd.memset(t[:, blk, PH - 1, :], 0.0)
                    nc.gpsimd.memset(t[:, blk, 0:PH - 1, PW - 2:PW], 0.0)
                    nc.gpsimd.memset(t[:, blk, 1:PH, 0:2], 0.0)

        def xload_dma(s):
            """Eight 16-row eighth DMAs covering both ci-blocks of x[s] —
            small chunks shorten the pooled tail (last cast is short)."""
            with nc.named_scope(f"xload{s}"):
                pooled.append(smallp.tile([128, 8], F32, tag="pooled",
                                          name=f"pooled{s}"))
                tiles = []
                for e in range(8):
                    blk, hh = e // 4, e % 4
                    xq = xqp.tile([128, H // 4, W], F32, tag="xq",
                                  name=f"xq{s}_{e}")
                    nc.sync.dma_start(
                        xq[:], x_d[s, blk * 128:(blk + 1) * 128,
                                   hh * 16:(hh + 1) * 16])
                    tiles.append(xq)
                return tiles

        def xload_cast(s, tiles):
            """hi cast (fused pooled accumulation) + lo residual.
            s0: hi on ACT, lo on DVE (conv needs them early);
            s1: hi on DVE, lo on the otherwise-idle gpsimd."""
            with nc.named_scope(f"xcast{s}"):
                for e, xq in enumerate(tiles):
                    blk, hh = e // 4, e % 4
                    r0 = 1 + 16 * hh
                    hi = xb[s][0][:, blk, r0:r0 + 16, 2:W + 2]
                    lo = xb[s][1][:, blk, r0:r0 + 16, 2:W + 2]
                    if s == 0:
                        nc.scalar.activation(
                            hi, xq[:], ACT_COPY,
                            accum_out=pooled[s][:, e:e + 1])
                        nc.vector.tensor_tensor(lo, xq[:], hi, SUB)
                    else:
                        nc.vector.tensor_scalar(
                            hi, xq[:], 1.0, None, MULT, ADD,
                            accum_out=pooled[s][:, e:e + 1])
                        nc.gpsimd.tensor_tensor(lo, xq[:], hi, SUB)

        def params_rest():
            with nc.named_scope("params"):
                fc1n = constp.tile([128, C], F32)  # rows 0..64 = fc1_w
                nc.sync.dma_start(fc1n[0:HID, :], fc1_d[:])
                fc2n = constp.tile([128, 8, HID], F32)  # [i_in_blk, i_blk, j]
                nc.sync.dma_start(
                    fc2n[:], bass.AP(fc2_d, 0, [[HID, 128], [128 * HID, 8],
                                                [1, HID]]))
                fc1t = constp.tile([128, 2, HID], F32)  # [ci_in_blk, ci_blk, j]
                for blk in range(2):
                    tps = pstp.tile([128, HID], F32, tag="pt", name=f"tp1_{blk}")
                    nc.tensor.transpose(tps[:],
                                        fc1n[0:HID, blk * 128:(blk + 1) * 128],
                                        ident32[0:HID, 0:HID])
                    nc.scalar.copy(fc1t[:, blk, :], tps[:])
                fc2t = constp.tile([128, K * O], F32)  # unused rows 66..127
                # rows 0..64 = fc2_w.T ; row 65 = fc2_b (bias folded in)
                for half in range(2):
                    tps = pstp.tile([128, 512], F32, tag="pt",
                                    name=f"tp2_{half}")
                    for c in range(4):
                        nc.tensor.transpose(tps[0:HID, c * 128:(c + 1) * 128],
                                            fc2n[:, half * 4 + c, :],
                                            ident32[:])
                    nc.scalar.copy(fc2t[0:HID, half * 512:(half + 1) * 512],
                                   tps[0:HID, :])
                nc.sync.dma_start(fc2t[HID:HID + 1, :], fc2b_d[:].unsqueeze(0))
                return fc1t, fc2t

        def se_chain(s, fc1t, fc2t):
            with nc.named_scope(f"se{s}"):
                z_ps = pstp.tile([128, 1], F32, tag="pt", name=f"z{s}")
                cols = zcols[s]
                for i, (blk, col) in enumerate(cols):
                    nc.tensor.matmul(z_ps[0:HID, :], fc1t[:, blk, :],
                                     pooled[s][:, col:col + 1],
                                     start=(i == 0), stop=(i == len(cols) - 1))
                h_ext = smallp.tile([128, 1], F32, tag="hext", name=f"hext{s}")
                nc.vector.memset(h_ext[:], 1.0)  # row 65 stays 1.0 (bias row)
                # relu(z/4096): mean folded via scale (relu is scale-invariant)
                nc.scalar.activation(h_ext[0:HID, :], z_ps[0:HID, :], ACT_RELU,
                                     scale=1.0 / (H * W))
                y_ps = pstp.tile([128, K * 2], F32, tag="pt", name=f"y{s}")
                for c in range(K * 2):
                    nc.tensor.matmul(y_ps[:, c:c + 1],
                                     fc2t[0:HID + 1, c * 128:(c + 1) * 128],
                                     h_ext[0:HID + 1, :], start=True, stop=True)
                e = smallp.tile([128, K, 2], F32, tag="e", name=f"e{s}")
                nc.scalar.activation(e[:].rearrange("p a b -> p (a b)"),
                                     y_ps[:], ACT_EXP, scale=1.0 / TEMP)
                ssum = smallp.tile([128, 2], F32, tag="ssum", name=f"ssum{s}")
                er = e[:].rearrange("p k o -> p o k")
                nc.vector.tensor_reduce(ssum[:], er, mybir.AxisListType.X, ADD)
                rinv = smallp.tile([128, 2], F32, tag="rinv", name=f"rinv{s}")
                nc.vector.reciprocal(rinv[:], ssum[:])
                prob = smallp.tile([128, 2, K], F32, tag="prob", name=f"prob{s}")
                for ob in range(2):
                    nc.vector.tensor_scalar_mul(prob[:, ob], er[:, ob],
                                                rinv[:, ob:ob + 1])
                return prob

        wb = [wbank.tile([128, K, C, NOFF], BF16, name=f"wb{ob}")
              for ob in range(2)]

        def load_w_dma(ob):
            # ci-half-major chunks so agg/transposes for ci-block 0 can
            # start while ci-block 1 is still in flight on the DMA ring.
            # Returns the f32 staging tiles: sample 0's agg reads them
            # directly (skips the cast latency on its critical path); the
            # bf16 wb copy only feeds sample 1's agg.
            staged = {}
            with nc.named_scope(f"wload{ob}"):
                for cb in range(2):
                    for k in range(K):
                        wst = wstage.tile([128, CF // 2], F32, tag="wst")
                        nc.sync.dma_start(
                            wst[:],
                            w_d[k, ob * 128:(ob + 1) * 128,
                                cb * 128:(cb + 1) * 128].rearrange(
                                    "p c a b -> p (c a b)"))
                        staged[(cb, k)] = wst
            return staged

        def cast_w(ob, staged):
            # wb only feeds sample 1's agg (late deadline): cast on the
            # idle gpsimd, except the first group which fits on ACT early.
            with nc.named_scope(f"wcast{ob}"):
                for cb in range(2):
                    for k in range(K):
                        dst = wb[ob][:, k, cb * 128:(cb + 1) * 128,
                                     :].rearrange("p c o -> p (c o)")
                        if ob == 0 and cb == 0:
                            nc.scalar.copy(dst, staged[(cb, k)][:])
                        else:
                            nc.gpsimd.tensor_copy(dst, staged[(cb, k)][:])

        # agg (bf16, [o, ci, off]) on DVE; emitted separately from the PE
        # transposes + hi/lo fp8 quant (aggt[set] [ci_in_blk, off, ci_blk, o])
        # so the DVE chain pipelines with the W DMA chunk arrivals and the
        # transpose/quant latency hides under unrelated conv matmuls.
        def agg_compute(s, ob, agg, staged=None):
            with nc.named_scope(f"agg{s}_{ob}"):
                for cb in range(2):
                    asl = agg[ob][:, cb * 128:(cb + 1) * 128,
                                  :].rearrange("p c o -> p (c o)")
                    for k in range(K):
                        if staged is not None:  # f32 staging (sample 0)
                            src = staged[(cb, k)][:]
                        else:  # resident bf16 bank (sample 1)
                            src = wb[ob][:, k, cb * 128:(cb + 1) * 128,
                                         :].rearrange("p c o -> p (c o)")
                        sc = se[s][:, ob, k:k + 1]
                        if k == 0:
                            nc.vector.tensor_scalar_mul(asl, src, sc)
                        else:
                            nc.vector.scalar_tensor_tensor(asl, src, sc,
                                                           asl, MULT, ADD)

        def agg_transpose(s, ob, agg, aggt):
            with nc.named_scope(f"transp{s}_{ob}"):
                for cb in range(2):
                    for gi, (o0, o1) in enumerate(TGROUPS):
                        n = o1 - o0
                        pt = pstp.tile([128, 4, 128], BF16, tag="pt",
                                       name=f"pt{s}_{ob}_{cb}_{gi}")
                        for oi in range(n):
                            nc.tensor.transpose(
                                pt[:, oi, :],
                                agg[ob][:, cb * 128:(cb + 1) * 128, o0 + oi],
                                ident[:])
                        src = pt[:, 0:n, :]
                        dst_hi = aggt[0][:, o0:o1, cb, ob * 128:(ob + 1) * 128]
                        dst_lo = aggt[1][:, o0:o1, cb, ob * 128:(ob + 1) * 128]
                        nc.scalar.activation(dst_hi, src, ACT_COPY,
                                             scale=WSCALE)
                        nc.vector.scalar_tensor_tensor(dst_lo, src, WSCALE,
                                                       dst_hi, MULT, SUB)

        def agg_transpose2(s, ob, agg, aggt):
            """Variant: one 8-off transpose batch + one single-off batch per
            ci-block -> only 2 hi + 2 lo quant ops per (s, ob, cb)."""
            with nc.named_scope(f"transp{s}_{ob}"):
                for cb in range(2):
                    for gi, (o0, o1) in enumerate(((0, 8), (8, 9))):
                        n = o1 - o0
                        pt = pstp.tile([128, 8, 128], BF16, tag="pt",
                                       name=f"pt{s}_{ob}_{cb}_{gi}")
                        for oi in range(n):
                            nc.tensor.transpose(
                                pt[:, oi, :],
                                agg[ob][:, cb * 128:(cb + 1) * 128, o0 + oi],
                                ident[:])
                        src = pt[:, 0:n, :]
                        dst_hi = aggt[0][:, o0:o1, cb, ob * 128:(ob + 1) * 128]
                        dst_lo = aggt[1][:, o0:o1, cb, ob * 128:(ob + 1) * 128]
                        nc.scalar.activation(dst_hi, src, ACT_COPY,
                                             scale=WSCALE)
                        nc.vector.scalar_tensor_tensor(dst_lo, src, WSCALE,
                                                       dst_hi, MULT, SUB)

        # DoubleRow pairings: four within-ci-block OFF pairs (so those
        # matmuls depend on only one ci-block's aggt) + off8 paired across
        # ci-blocks. Same matmul count/cost as pure cb-pairing, but conv can
        # start as soon as ci-block 0's weights are quantized.
        OFFPAIRS = ((0, 1), (3, 4), (6, 7), (2, 5))

        def strides_of(ap):
            apl = [list(p) for p in ap.ap]
            return apl

        def conv(s, ob, aggt, mid=None):
            out_hw = out_d[s].rearrange("o a b -> o (a b)")
            xfull = [xb[s][v][:] for v in range(2)]
            xst = [strides_of(a) for a in xfull]  # [[sp,128],[scb,2],[sr,PH],[sc,PW]]
            afull = [aggt[v][:] for v in range(2)]
            ast = [strides_of(a) for a in afull]  # [[sp,128],[soff,9],[scb,2],[so,256]]
            with nc.named_scope(f"conv{s}_{ob}"):
                c0 = 0
                for ci, csz in enumerate(HWCHUNKS):
                    if ci == 0 and mid is not None:
                        mid()
                    pc = pscp.tile([128, max(HWCHUNKS)], F32, tag="conv",
                                   name=f"conv{s}_{ob}_{ci}")
                    for b in range(csz // 512):
                        h0 = (c0 + b * 512) // W
                        n_mm = 0
                        # cb-major then set-major: cb0 hi*hi first
                        for cb in range(2):
                            for (ws, xs_i) in PRODUCTS:
                                sp, scb, sr, sc = (xst[xs_i][0][0],
                                                   xst[xs_i][1][0],
                                                   xst[xs_i][2][0],
                                                   xst[xs_i][3][0])
                                asp, soff, sacb, so = (ast[ws][0][0],
                                                       ast[ws][1][0],
                                                       ast[ws][2][0],
                                                       ast[ws][3][0])
                                for (o1, o2) in OFFPAIRS:
                                    dh1, dw1 = o1 // KK - 1, o1 % KK - 1
                                    dh2, dw2 = o2 // KK - 1, o2 % KK - 1
                                    delta = (dh2 - dh1) * sr + (dw2 - dw1) * sc
                                    lhsT = bass.AP(
                                        afull[ws].tensor,
                                        afull[ws].offset + o1 * soff
                                        + cb * sacb + ob * 128 * so,
                                        [[asp, 128], [(o2 - o1) * soff, 2],
                                         [so, 128]])
                                    for r in range(8):
                                        roff = (xfull[xs_i].offset + cb * scb
                                                + (h0 + r + 1 + dh1) * sr
                                                + (2 + dw1) * sc)
                                        rhs = bass.AP(
                                            xfull[xs_i].tensor, roff,
                                            [[sp, 128], [delta, 2], [sc, W]])
                                        o0 = b * 512 + r * 64
                                        nc.tensor.matmul(
                                            pc[:, o0:o0 + 64], lhsT, rhs,
                                            start=(n_mm == 0),
                                            stop=False, perf_mode=DR)
                                        n_mm += 1
                        # off8 (dh=+1, dw=+1): pair the two ci-blocks
                        for (ws, xs_i) in PRODUCTS:
                            xt = xb[s][xs_i]
                            lhsT = aggt[ws][:, 8, :, ob * 128:(ob + 1) * 128]
                            for r in range(8):
                                rhs = xt[:, :, h0 + r + 2, 3:3 + W]
                                o0 = b * 512 + r * 64
                                nc.tensor.matmul(
                                    pc[:, o0:o0 + 64], lhsT, rhs,
                                    start=False,
                                    stop=(n_mm == 27 * 8 - 1),
                                    perf_mode=DR)
                                n_mm += 1
                    ost = ostp.tile([128, max(HWCHUNKS)], F32, tag="ost")
                    nc.scalar.activation(ost[:, 0:csz], pc[:, 0:csz],
                                         ACT_COPY, scale=1.0 / WSCALE)
                    nc.sync.dma_start(
                        out_hw[ob * 128:(ob + 1) * 128, c0:c0 + csz],
                        ost[:, 0:csz])
                    c0 += csz

        # ---- emission: DMA order x_s0, fc1, fc2, W0, W1, x_s1, outs ------
        # s1's transposes/quant are emitted mid-way through the preceding
        # conv block so their latency hides under ready conv matmuls.
        halo_memset(0)
        xq0 = xload_dma(0)
        xload_cast(0, xq0)
        fc1t, fc2t = params_rest()
        agg0 = [aggp.tile([128, C, NOFF], BF16, tag="agg", name=f"agg0_{ob}")
                for ob in range(2)]
        aggt0 = [aggtp.tile([128, NOFF, 2, O], FP8, name=f"aggt0_{v}")
                 for v in range(2)]
        with tc.high_priority():
            se.append(se_chain(0, fc1t, fc2t))
        wst0 = load_w_dma(0)
        agg_compute(0, 0, agg0, wst0)
        agg_transpose(0, 0, agg0, aggt0)
        cast_w(0, wst0)
        wst1 = load_w_dma(1)
        agg_compute(0, 1, agg0, wst1)
        agg_transpose(0, 1, agg0, aggt0)
        cast_w(1, wst1)
        halo_memset(1)
        xq1 = xload_dma(1)
        conv(0, 0, aggt0)
        xload_cast(1, xq1)
        se.append(se_chain(1, fc1t, fc2t))
        agg1 = [aggp.tile([128, C, NOFF], BF16, tag="agg", name=f"agg1_{ob}")
                for ob in range(2)]
        aggt1 = [aggtp.tile([128, NOFF, 2, O], FP8, name=f"aggt1_{v}")
                 for v in range(2)]
        agg_compute(1, 0, agg1)
        agg_compute(1, 1, agg1)
        conv(0, 1, aggt0, mid=lambda: agg_transpose(1, 0, agg1, aggt1))
        conv(1, 0, aggt1, mid=lambda: agg_transpose(1, 1, agg1, aggt1))
        conv(1, 1, aggt1)


_NC_CACHE = None


def _get_nc():
    global _NC_CACHE
    if _NC_CACHE is None:
        _NC_CACHE = build_kernel()
    return _NC_CACHE


def make_in_maps(x, fc1_w, fc2_w, fc2_b, weight):
    x = np.ascontiguousarray(x, dtype=np.float32)
    shared = {
        "fc1_w": np.ascontiguousarray(fc1_w, dtype=np.float32),
        "fc2_w": np.ascontiguousarray(fc2_w, dtype=np.float32),
        "fc2_b": np.ascontiguousarray(fc2_b, dtype=np.float32),
        "weight": np.ascontiguousarray(weight, dtype=np.float32),
    }
    return [{"x": x[c * BS:(c + 1) * BS], **shared} for c in range(N_CORES)]


def kernel(x, fc1_w, fc2_w, fc2_b, weight):
    import time
    nc = _get_nc()
    in_maps = make_in_maps(x, fc1_w, fc2_w, fc2_b, weight)
    res = None
    for attempt in range(3):
        try:
            res = run_bass_kernel_spmd(nc, in_maps,
                                       core_ids=list(range(N_CORES)))
            break
        except Exception:
            # transient device wedge (NRT_EXEC_UNIT_UNRECOVERABLE); the
            # axon terminal recovers after a short wait
            if attempt == 2:
                raise
            time.sleep(60 * (attempt + 1))
    return np.concatenate([res.results[c]["out"] for c in range(N_CORES)],
                          axis=0).astype(np.float32)
